# revision 39
# baseline (speedup 1.0000x reference)
"""MelSpectrogram Trainium2 kernel.

Full inputs in, full output out. Data-parallel over batch B=8 across the
8 NeuronCores (one audio row per core); DFT basis and mel filterbank are
replicated (prepped host-side into matmul-friendly layouts).

Per-core device algorithm (one audio row, T=1048576):

The reflect-padded signal x (len 4099*256) is split host-side into even/
odd sample streams laid out as SBUF tensors SE/SO [128, 4099] with
SE[l,j] = x[2*(j*128+l)], SO[l,j] = x[2*(j*128+l)+1]. Because HOP=256
divides FILTER_LEN=1024, frame f's even-tap block p (taps 2*(p*128+l))
is SE[:, f+p] -- the windowed DFT needs no frame materialization, just
shifted rhs slices.

Radix-2 bin pairing: the DFT basis satisfies basis[512-k, n] =
+/-(-1)^n basis[k, n], so with P = Ce@xe, Q = Co@xo, V = Se@xe,
W = So@xo (even/odd tap split of the windowed cos/sin bases, bin rows
1..256):
    mag[k]      = sqrt((P+Q)^2 + (V+W)^2)   k = 1..256
    mag[512-k]  = sqrt((P-Q)^2 + (V-W)^2)   k = 1..255
Mel filter weights at bins 0 and 512 are exactly zero (fmin=0,
fmax=sr/2 edge filters), so bins 1..511 cover everything: the DFT
matmul work halves versus the direct form, in clean 2x128-row tiles
(the bin-256 duplicate on the B side is zeroed in the permuted mel
matrix host-side).

magnitudes feed a PSUM-accumulated mel projection on PE; final
log1p(1e4*x) = Ln(1e4*x + 1) on ACT, streamed against the output DMA.
"""

import os
import sys

sys.path.insert(0, "/opt/trn_rl_repo")

import numpy as np
import concourse.bass as bass
import concourse.mybir as mybir
import concourse.tile as tile
from concourse.bass_utils import run_bass_kernel_spmd
from concourse.vector_clock import ScopedClock

N_CORES = 8
T = 1048576
PAD = 384
SEG = 4099  # (T + 2*PAD) / 256
F = 4096  # output frames
NT = 512  # frames per tile
N_TILES = F // NT
DT = mybir.dt.float16
NP_DT = np.float16

_cache = {}


class _PatchedTileContext(tile.TileContext):
    # This walrus build rejects >1 sync-wait per instruction (and any on
    # the kernel-tail Drain): carry the global-clock waits on
    # one-NoOp-per-wait ahead of the drain instead.
    def _drain_and_barrier(self, tick_clock, wait_clock):
        nop_inst = self.nc.sync.nop(nofuse=True, hint="pre_drain_waits")
        wait_clock.add_sem_waits(
            nop_inst.ins, ScopedClock({None: tick_clock.global_clock})
        )
        waits = list(nop_inst.ins.sync_info.on_wait)
        if len(waits) > 1:
            si = nop_inst.ins.sync_info
            si.on_wait = waits[:1]
            nop_inst.ins.sync_info = si
            for w in waits[1:]:
                extra = self.nc.sync.nop(nofuse=True, hint="pre_drain_waits")
                esi = extra.ins.sync_info or mybir.SyncInfo(on_wait=[], on_update=[])
                esi.on_wait = [w]
                extra.ins.sync_info = esi
        self.nc.sync.drain()
        self.nc.all_engine_barrier()
        assert self.sems is not None
        popped = self.nc._tile_sem_poison_stack.pop()
        assert popped is self._sem_poison
        self.nc.clear_and_free_semaphores(list(self.sems.allocated().values()))
        self.nc.all_engine_barrier()


def _split_sync_waits(nc, cap=1):
    # Hoist excess sync-waits onto same-engine NoOps placed just before
    # the instruction (engines are in-order, so semantics preserved).
    for f in nc.m.functions:
        for bb in f.blocks:
            out = []
            changed = False
            for inst in bb.instructions:
                si = inst.sync_info
                waits = list(si.on_wait) if si else []
                if len(waits) > cap:
                    changed = True
                    for w in waits[:-cap]:
                        nop = mybir.InstNoOp(
                            name=nc.get_next_instruction_name(), ins=[], outs=[]
                        )
                        nop.engine = inst.engine
                        nop.sync_info = mybir.SyncInfo(on_wait=[w], on_update=[])
                        out.append(nop)
                    si.on_wait = waits[-cap:]
                    inst.sync_info = si
                out.append(inst)
            if changed:
                bb.instructions = out


def _build_program():
    nc = bass.Bass()
    se_d = nc.dram_tensor("se", [128, SEG], DT, kind="ExternalInput")
    so_d = nc.dram_tensor("so", [128, SEG], DT, kind="ExternalInput")
    # 32 blocks of [128,128]: q = m*16 + p*4 + x, x in (Ce,Co,Se,So)
    wts_d = nc.dram_tensor("wts", [128, 32 * 128], DT, kind="ExternalInput")
    melt_d = nc.dram_tensor("melt", [128, 4 * 80], DT, kind="ExternalInput")
    out_d = nc.dram_tensor("out", [80, F], mybir.dt.float32, kind="ExternalOutput")

    f32 = mybir.dt.float32
    NT2 = 2 * NT
    with _PatchedTileContext(nc) as tc:
        with (
            tc.tile_pool(name="const", bufs=1) as const,
            tc.tile_pool(name="work", bufs=2) as work,
            tc.tile_pool(name="psum", bufs=1, space="PSUM") as pp,
        ):
            se_c, so_c = [], []
            for t in range(N_TILES):
                c0 = t * NT
                c1 = min(SEG, (t + 1) * NT + 3)
                se = const.tile([128, c1 - c0], DT, tag=f"se{t}", name=f"se{t}")
                so = const.tile([128, c1 - c0], DT, tag=f"so{t}", name=f"so{t}")
                se_c.append(se)
                so_c.append(so)
            # weights split per (m, p) so matmuls gate on 128KB pieces that
            # stream in program order down SP's FIFO DMA queue
            wp = [
                const.tile([128, 512], DT, tag=f"wp{j}", name=f"wp{j}")
                for j in range(8)
            ]
            melt_sb = const.tile([128, 4 * 80], DT)

            # SP FIFO delivers in emission order: first-tile criticals,
            # then the rest of the weights, then audio chunks in use order.
            nc.sync.dma_start(out=wp[0][:], in_=wts_d[:, 0:512])
            nc.sync.dma_start(out=se_c[0][:], in_=se_d[:, 0 : NT + 3])
            nc.sync.dma_start(out=so_c[0][:], in_=so_d[:, 0 : NT + 3])
            for j in range(1, 8):
                nc.sync.dma_start(out=wp[j][:], in_=wts_d[:, j * 512 : (j + 1) * 512])
            nc.sync.dma_start(out=melt_sb[:], in_=melt_d[:])
            for t in range(1, N_TILES):
                c0 = t * NT
                c1 = min(SEG, (t + 1) * NT + 3)
                nc.sync.dma_start(out=se_c[t][:], in_=se_d[:, c0:c1])
                nc.sync.dma_start(out=so_c[t][:], in_=so_d[:, c0:c1])
            outbuf = const.tile([80, F], f32)

            def lhsT(m, p, x):
                return wp[m * 4 + p][:, x * 128 : (x + 1) * 128]

            BP = mybir.AluOpType.bypass
            ADD = mybir.AluOpType.add
            SUB = mybir.AluOpType.subtract
            MUL = mybir.AluOpType.mult

            # Software pipeline: engines have strict in-order queues, so an
            # op whose input is produced in the same tile stalls its whole
            # queue. Emitting each stage one tile later than its producer
            # ensures inputs are a full tile old by the time the engine
            # reaches the op. Stages:
            #   S0(t): DFT matmuls + PSUM-freeing copies/combines
            #   S1(t-1): squares   S2(t-2): adds   S3(t-3): sqrts
            #   S4(t-4): mel matmuls + outbuf copy
            s1q, s2q, s3q, s4q = [], [], [], []

            # HAM warmup: dummy matmuls on uninitialized SBUF (no DMA
            # deps, output never read) keep the PE busy while the input
            # DMAs stream, so the real matmuls start at 2.4GHz instead of
            # paying the ~3.4us cold-clock window
            warmsrc = const.tile([128, NT], DT, name="warmsrc")
            nc.gpsimd.memset(warmsrc[:], 0)
            warm_ps = pp.tile([80, NT], f32, tag="mel", bufs=1, name="warm")
            for _ in range(12):
                nc.tensor.matmul(
                    warm_ps[:], warmsrc[:, :80], warmsrc[:], start=True, stop=True
                )

            def _emit_s1(it):
                u, v, un, vn, t0p = it
                a = work.tile([128, NT2], DT, tag="a", bufs=3, name=f"a{t0p}")
                nc.gpsimd.tensor_tensor(a[:], u[:], u[:], MUL)
                b = work.tile([128, NT2], DT, tag="b", bufs=3, name=f"b{t0p}")
                nc.scalar.square(b[:], v[:])
                a2 = work.tile([128, NT2], DT, tag="a2", bufs=3, name=f"a2_{t0p}")
                nc.vector.scalar_tensor_tensor(a2[:], un[:], 1.0, un[:], MUL, MUL)
                b2 = work.tile([128, NT2], DT, tag="b2", bufs=3, name=f"b2_{t0p}")
                nc.gpsimd.tensor_tensor(b2[:], vn[:], vn[:], MUL)
                s2q.append((a, b, a2, b2, t0p))

            def _emit_s2(it):
                a, b, a2, b2, t0p = it
                m2A = work.tile([128, NT2], DT, tag="m2A", bufs=3, name=f"m2A{t0p}")
                nc.gpsimd.tensor_tensor(m2A[:], a[:], b[:], ADD)
                m2B = work.tile([128, NT2], DT, tag="m2B", bufs=3, name=f"m2B{t0p}")
                nc.gpsimd.tensor_tensor(m2B[:], a2[:], b2[:], ADD)
                s3q.append((m2A, m2B, t0p))

            def _emit_s3(it):
                m2A, m2B, t0p = it
                magA = work.tile([128, NT2], DT, tag="magA", bufs=3, name=f"mgA{t0p}")
                nc.scalar.sqrt(magA[:], m2A[:])
                magB = work.tile([128, NT2], DT, tag="magB", bufs=3, name=f"mgB{t0p}")
                nc.scalar.sqrt(magB[:], m2B[:])
                s4q.append((magA, magB, t0p))

            def _emit_s4(it):
                mA, mB, t0p = it
                mel_ps = pp.tile([80, NT], f32, tag="mel", bufs=1, name=f"mel{t0p}")
                rhss = [
                    mA[:, 0:NT], mA[:, NT:NT2],
                    mB[:, 0:NT], mB[:, NT:NT2],
                ]
                for i, rhs in enumerate(rhss):
                    nc.tensor.matmul(
                        mel_ps[:], melt_sb[:, i * 80 : (i + 1) * 80], rhs,
                        start=(i == 0), stop=(i == 3),
                    )
                nc.vector.tensor_copy(out=outbuf[:, t0p : t0p + NT], in_=mel_ps[:])

            for t in range(N_TILES):
                t0 = t * NT
                # narrow per-m PSUM tiles (P,Q,V double-buffered; W and mel
                # single); wide SBUF intermediates written in m-halves
                qs = work.tile([128, NT2], DT, tag="qs", bufs=4)
                ws = work.tile([128, NT2], DT, tag="ws", bufs=4)
                u = work.tile([128, NT2], DT, tag="u", bufs=4)
                un = work.tile([128, NT2], DT, tag="un", bufs=4)
                v = work.tile([128, NT2], DT, tag="v", bufs=4)
                vn = work.tile([128, NT2], DT, tag="vn", bufs=4)
                for m in range(2):
                    sl = slice(m * NT, (m + 1) * NT)
                    P = pp.tile([128, NT], f32, tag="P", name=f"P{t}_{m}")
                    Q = pp.tile([128, NT], f32, tag="Q", name=f"Q{t}_{m}")
                    V = pp.tile([128, NT], f32, tag="V", name=f"V{t}_{m}")
                    W = pp.tile([128, NT], f32, tag="W", bufs=1, name=f"W{t}_{m}")
                    for p in range(4):
                        rhsE = se_c[t][:, p : p + NT]
                        rhsO = so_c[t][:, p : p + NT]
                        st = dict(start=(p == 0), stop=(p == 3))
                        nc.tensor.matmul(P[:], lhsT(m, p, 0), rhsE, **st)
                        nc.tensor.matmul(Q[:], lhsT(m, p, 1), rhsO, **st)
                        nc.tensor.matmul(V[:], lhsT(m, p, 2), rhsE, **st)
                        nc.tensor.matmul(W[:], lhsT(m, p, 3), rhsO, **st)
                    nc.scalar.copy(qs[:, sl], Q[:])
                    nc.scalar.copy(ws[:, sl], W[:])
                    nc.vector.scalar_tensor_tensor(
                        u[:, sl], P[:], 0.0, qs[:, sl], BP, ADD
                    )
                    nc.vector.scalar_tensor_tensor(
                        un[:, sl], P[:], -1.0, qs[:, sl], MUL, ADD
                    )
                    nc.vector.scalar_tensor_tensor(
                        v[:, sl], V[:], 0.0, ws[:, sl], BP, ADD
                    )
                    nc.vector.scalar_tensor_tensor(
                        vn[:, sl], V[:], -1.0, ws[:, sl], MUL, ADD
                    )
                # A side (bins 1..256): mag = sqrt((P+Q)^2 + (V+W)^2)
                # B side (bins 511..257): mag = sqrt((Q-P)^2 + (W-V)^2)
                s1q.append((u, v, un, vn, t0))
                if len(s1q) > 1:
                    _emit_s1(s1q.pop(0))
                if len(s2q) > 1:
                    _emit_s2(s2q.pop(0))
                if len(s3q) > 1:
                    _emit_s3(s3q.pop(0))
                if len(s4q) > 1:
                    _emit_s4(s4q.pop(0))

            for _ in range(N_TILES):
                if s1q:
                    _emit_s1(s1q.pop(0))
                if s2q:
                    _emit_s2(s2q.pop(0))
                if s3q:
                    _emit_s3(s3q.pop(0))
                if s4q:
                    _emit_s4(s4q.pop(0))

            outln = const.tile([80, F], f32)
            H = F // 4
            for h in range(4):
                nc.scalar.activation(
                    outln[:, h * H : (h + 1) * H], outbuf[:, h * H : (h + 1) * H],
                    mybir.ActivationFunctionType.Ln, bias=1.0, scale=10000.0,
                )
                nc.sync.dma_start(
                    out=out_d[:, h * H : (h + 1) * H],
                    in_=outln[:, h * H : (h + 1) * H],
                )
    _split_sync_waits(nc)
    return nc


def _prep_inputs(audio, basis_r, basis_i, mel_basis):
    audio = np.asarray(audio, dtype=np.float32)
    basis_r = np.asarray(basis_r, dtype=np.float32)
    basis_i = np.asarray(basis_i, dtype=np.float32)
    mel_basis = np.asarray(mel_basis, dtype=np.float32)

    Ce = basis_r[:257, 0::2]
    Co = basis_r[:257, 1::2]
    Se = basis_i[:257, 0::2]
    So = basis_i[:257, 1::2]
    mats = (Ce, Co, Se, So)
    wts = np.empty((128, 32 * 128), dtype=NP_DT)
    for m in range(2):
        for p in range(4):
            for x in range(4):
                q = (m * 16 + p * 4 + x) * 128
                blk = mats[x][1 + m * 128 : 1 + (m + 1) * 128, p * 128 : (p + 1) * 128]
                wts[:, q : q + 128] = blk.T.astype(NP_DT)

    # mel matrix, bins permuted to the device's mag row order:
    # A side rows = bins 1..256; B side row j = bin 511-j (row 255 is the
    # bin-256 duplicate -> zeroed)
    melA = mel_basis[:, 1:257]
    melB = mel_basis[:, [511 - j for j in range(256)]].copy()
    melB[:, 255] = 0.0
    melt = np.empty((128, 4 * 80), dtype=NP_DT)
    melt[:, 0:80] = melA[:, 0:128].T.astype(NP_DT)
    melt[:, 80:160] = melA[:, 128:256].T.astype(NP_DT)
    melt[:, 160:240] = melB[:, 0:128].T.astype(NP_DT)
    melt[:, 240:320] = melB[:, 128:256].T.astype(NP_DT)

    in_maps = []
    for b in range(N_CORES):
        row = audio[b]
        x = np.concatenate([row[PAD:0:-1], row, row[-2 : -PAD - 2 : -1]])
        se = np.ascontiguousarray(x[0::2].reshape(SEG, 128).T).astype(NP_DT)
        so = np.ascontiguousarray(x[1::2].reshape(SEG, 128).T).astype(NP_DT)
        in_maps.append({"se": se, "so": so, "wts": wts, "melt": melt})
    return in_maps


def kernel(audio, basis_r, basis_i, mel_basis):
    if "nc" not in _cache:
        _cache["nc"] = _build_program()
    nc = _cache["nc"]
    in_maps = _prep_inputs(audio, basis_r, basis_i, mel_basis)

    trace = os.environ.get("MELSPEC_TRACE") == "1"
    if trace:
        import types
        import trn_agent_boot.trn_boot as tb
        import concourse.bass_utils as bu

        if "antenv.axon_hooks" not in sys.modules:
            hook = tb._ntff_profile_via_ctypes("/opt/axon/libaxon_pjrt.so")
            mod = types.ModuleType("antenv.axon_hooks")
            mod.get_axon_ntff_profile_hook = lambda: hook
            sys.modules["antenv.axon_hooks"] = mod
        bu.upload_artifacts = lambda tmpdir: f"local://{tmpdir}"

    res = run_bass_kernel_spmd(nc, in_maps, list(range(N_CORES)), trace=trace)
    _cache["last_results"] = res
    out = np.stack([res.results[i]["out"] for i in range(N_CORES)])
    return out.astype(np.float32)


# revision 40
# speedup vs baseline: 1.1101x; 1.1101x over previous
"""MelSpectrogram Trainium2 kernel.

Full inputs in, full output out. Data-parallel over batch B=8 across the
8 NeuronCores (one audio row per core); DFT basis and mel filterbank are
replicated (prepped host-side into matmul-friendly layouts).

Per-core device algorithm (one audio row, T=1048576):

The reflect-padded signal x (len 4099*256) is split host-side into even/
odd sample streams laid out as SBUF tensors SE/SO [128, 4099] with
SE[l,j] = x[2*(j*128+l)], SO[l,j] = x[2*(j*128+l)+1]. Because HOP=256
divides FILTER_LEN=1024, frame f's even-tap block p (taps 2*(p*128+l))
is SE[:, f+p] -- the windowed DFT needs no frame materialization, just
shifted rhs slices.

Radix-2 bin pairing: the DFT basis satisfies basis[512-k, n] =
+/-(-1)^n basis[k, n], so with P = Ce@xe, Q = Co@xo, V = Se@xe,
W = So@xo (even/odd tap split of the windowed cos/sin bases, bin rows
1..256):
    mag[k]      = sqrt((P+Q)^2 + (V+W)^2)   k = 1..256
    mag[512-k]  = sqrt((P-Q)^2 + (V-W)^2)   k = 1..255
Mel filter weights at bins 0 and 512 are exactly zero (fmin=0,
fmax=sr/2 edge filters), so bins 1..511 cover everything: the DFT
matmul work halves versus the direct form, in clean 2x128-row tiles
(the bin-256 duplicate on the B side is zeroed in the permuted mel
matrix host-side).

magnitudes feed a PSUM-accumulated mel projection on PE; final
log1p(1e4*x) = Ln(1e4*x + 1) on ACT, streamed against the output DMA.
"""

import os
import sys

sys.path.insert(0, "/opt/trn_rl_repo")

import numpy as np
import concourse.bass as bass
import concourse.mybir as mybir
import concourse.tile as tile
from concourse.bass_utils import run_bass_kernel_spmd
from concourse.vector_clock import ScopedClock

N_CORES = 8
T = 1048576
PAD = 384
SEG = 4099  # (T + 2*PAD) / 256
F = 4096  # output frames
NT = 512  # frames per tile
N_TILES = F // NT
DT = mybir.dt.float16
NP_DT = np.float16

_cache = {}


class _PatchedTileContext(tile.TileContext):
    # This walrus build rejects >1 sync-wait per instruction (and any on
    # the kernel-tail Drain): carry the global-clock waits on
    # one-NoOp-per-wait ahead of the drain instead.
    def _drain_and_barrier(self, tick_clock, wait_clock):
        nop_inst = self.nc.sync.nop(nofuse=True, hint="pre_drain_waits")
        wait_clock.add_sem_waits(
            nop_inst.ins, ScopedClock({None: tick_clock.global_clock})
        )
        waits = list(nop_inst.ins.sync_info.on_wait)
        if len(waits) > 1:
            si = nop_inst.ins.sync_info
            si.on_wait = waits[:1]
            nop_inst.ins.sync_info = si
            for w in waits[1:]:
                extra = self.nc.sync.nop(nofuse=True, hint="pre_drain_waits")
                esi = extra.ins.sync_info or mybir.SyncInfo(on_wait=[], on_update=[])
                esi.on_wait = [w]
                extra.ins.sync_info = esi
        self.nc.sync.drain()
        self.nc.all_engine_barrier()
        assert self.sems is not None
        popped = self.nc._tile_sem_poison_stack.pop()
        assert popped is self._sem_poison
        self.nc.clear_and_free_semaphores(list(self.sems.allocated().values()))
        self.nc.all_engine_barrier()


def _split_sync_waits(nc, cap=1):
    # Hoist excess sync-waits onto same-engine NoOps placed just before
    # the instruction (engines are in-order, so semantics preserved).
    for f in nc.m.functions:
        for bb in f.blocks:
            out = []
            changed = False
            for inst in bb.instructions:
                si = inst.sync_info
                waits = list(si.on_wait) if si else []
                if len(waits) > cap:
                    changed = True
                    for w in waits[:-cap]:
                        nop = mybir.InstNoOp(
                            name=nc.get_next_instruction_name(), ins=[], outs=[]
                        )
                        nop.engine = inst.engine
                        nop.sync_info = mybir.SyncInfo(on_wait=[w], on_update=[])
                        out.append(nop)
                    si.on_wait = waits[-cap:]
                    inst.sync_info = si
                out.append(inst)
            if changed:
                bb.instructions = out


def _build_program():
    nc = bass.Bass()
    se_d = nc.dram_tensor("se", [128, SEG], DT, kind="ExternalInput")
    so_d = nc.dram_tensor("so", [128, SEG], DT, kind="ExternalInput")
    # 32 blocks of [128,128]: q = m*16 + p*4 + x, x in (Ce,Co,Se,So)
    wts_d = nc.dram_tensor("wts", [128, 32 * 128], DT, kind="ExternalInput")
    melt_d = nc.dram_tensor("melt", [128, 4 * 80], DT, kind="ExternalInput")
    out_d = nc.dram_tensor("out", [80, F], mybir.dt.float32, kind="ExternalOutput")

    f32 = mybir.dt.float32
    NT2 = 2 * NT
    with _PatchedTileContext(nc) as tc:
        with (
            tc.tile_pool(name="const", bufs=1) as const,
            tc.tile_pool(name="work", bufs=2) as work,
            tc.tile_pool(name="psum", bufs=1, space="PSUM") as pp,
        ):
            se_c, so_c = [], []
            for t in range(N_TILES):
                c0 = t * NT
                c1 = min(SEG, (t + 1) * NT + 3)
                se = const.tile([128, c1 - c0], DT, tag=f"se{t}", name=f"se{t}")
                so = const.tile([128, c1 - c0], DT, tag=f"so{t}", name=f"so{t}")
                se_c.append(se)
                so_c.append(so)
            # weights split per (m, p) so matmuls gate on 128KB pieces that
            # stream in program order down SP's FIFO DMA queue
            wp = [
                const.tile([128, 512], DT, tag=f"wp{j}", name=f"wp{j}")
                for j in range(8)
            ]
            melt_sb = const.tile([128, 4 * 80], DT)

            # SP FIFO delivers in emission order: first-tile criticals,
            # then the rest of the weights, then audio chunks in use order.
            nc.sync.dma_start(out=wp[0][:], in_=wts_d[:, 0:512])
            nc.sync.dma_start(out=se_c[0][:], in_=se_d[:, 0 : NT + 3])
            nc.sync.dma_start(out=so_c[0][:], in_=so_d[:, 0 : NT + 3])
            for j in range(1, 8):
                nc.sync.dma_start(out=wp[j][:], in_=wts_d[:, j * 512 : (j + 1) * 512])
            nc.sync.dma_start(out=melt_sb[:], in_=melt_d[:])
            for t in range(1, N_TILES):
                c0 = t * NT
                c1 = min(SEG, (t + 1) * NT + 3)
                nc.sync.dma_start(out=se_c[t][:], in_=se_d[:, c0:c1])
                nc.sync.dma_start(out=so_c[t][:], in_=so_d[:, c0:c1])
            outbuf = const.tile([80, F], f32)

            def lhsT(m, p, x):
                return wp[m * 4 + p][:, x * 128 : (x + 1) * 128]

            BP = mybir.AluOpType.bypass
            ADD = mybir.AluOpType.add
            SUB = mybir.AluOpType.subtract
            MUL = mybir.AluOpType.mult

            # Software pipeline: engines have strict in-order queues, so an
            # op whose input is produced in the same tile stalls its whole
            # queue. Emitting each stage one tile later than its producer
            # ensures inputs are a full tile old by the time the engine
            # reaches the op. Stages:
            #   S0(t): DFT matmuls + PSUM-freeing copies/combines
            #   S1(t-1): squares   S2(t-2): adds   S3(t-3): sqrts
            #   S4(t-4): mel matmuls + outbuf copy
            s1q, s2q, s3q, s4q = [], [], [], []

            # HAM warmup: dummy matmuls on uninitialized SBUF (no DMA
            # deps, output never read) keep the PE busy while the input
            # DMAs stream, so the real matmuls start at 2.4GHz instead of
            # paying the ~3.4us cold-clock window
            warmsrc = const.tile([128, NT], DT, name="warmsrc")
            nc.gpsimd.memset(warmsrc[:], 0)
            warm_ps = pp.tile([80, NT], f32, tag="mel", bufs=1, name="warm")
            for _ in range(12):
                nc.tensor.matmul(
                    warm_ps[:], warmsrc[:, :80], warmsrc[:], start=True, stop=True
                )

            def _emit_s1(it):
                u, v, un, vn, t0p = it
                a = work.tile([128, NT2], DT, tag="a", bufs=3, name=f"a{t0p}")
                nc.scalar.square(a[:], u[:])
                b = work.tile([128, NT2], DT, tag="b", bufs=3, name=f"b{t0p}")
                nc.scalar.square(b[:], v[:])
                a2 = work.tile([128, NT2], DT, tag="a2", bufs=3, name=f"a2_{t0p}")
                nc.vector.scalar_tensor_tensor(a2[:], un[:], 1.0, un[:], MUL, MUL)
                b2 = work.tile([128, NT2], DT, tag="b2", bufs=3, name=f"b2_{t0p}")
                nc.gpsimd.tensor_tensor(b2[:], vn[:], vn[:], MUL)
                s2q.append((a, b, a2, b2, t0p))

            def _emit_s2(it):
                a, b, a2, b2, t0p = it
                m2A = work.tile([128, NT2], DT, tag="m2A", bufs=3, name=f"m2A{t0p}")
                nc.gpsimd.tensor_tensor(m2A[:], a[:], b[:], ADD)
                m2B = work.tile([128, NT2], DT, tag="m2B", bufs=3, name=f"m2B{t0p}")
                nc.gpsimd.tensor_tensor(m2B[:], a2[:], b2[:], ADD)
                s3q.append((m2A, m2B, t0p))

            def _emit_s3(it):
                m2A, m2B, t0p = it
                magA = work.tile([128, NT2], DT, tag="magA", bufs=3, name=f"mgA{t0p}")
                nc.scalar.sqrt(magA[:], m2A[:])
                magB = work.tile([128, NT2], DT, tag="magB", bufs=3, name=f"mgB{t0p}")
                nc.scalar.sqrt(magB[:], m2B[:])
                s4q.append((magA, magB, t0p))

            def _emit_s4(it):
                mA, mB, t0p = it
                mel_ps = pp.tile([80, NT], f32, tag="mel", bufs=1, name=f"mel{t0p}")
                rhss = [
                    mA[:, 0:NT], mA[:, NT:NT2],
                    mB[:, 0:NT], mB[:, NT:NT2],
                ]
                for i, rhs in enumerate(rhss):
                    nc.tensor.matmul(
                        mel_ps[:], melt_sb[:, i * 80 : (i + 1) * 80], rhs,
                        start=(i == 0), stop=(i == 3),
                    )
                nc.vector.tensor_copy(out=outbuf[:, t0p : t0p + NT], in_=mel_ps[:])

            for t in range(N_TILES):
                t0 = t * NT
                # narrow per-m PSUM tiles (P,Q,V double-buffered; W and mel
                # single); wide SBUF intermediates written in m-halves
                qs = work.tile([128, NT2], DT, tag="qs", bufs=4)
                ws = work.tile([128, NT2], DT, tag="ws", bufs=4)
                u = work.tile([128, NT2], DT, tag="u", bufs=4)
                un = work.tile([128, NT2], DT, tag="un", bufs=4)
                v = work.tile([128, NT2], DT, tag="v", bufs=4)
                vn = work.tile([128, NT2], DT, tag="vn", bufs=4)
                for m in range(2):
                    sl = slice(m * NT, (m + 1) * NT)
                    P = pp.tile([128, NT], f32, tag="P", name=f"P{t}_{m}")
                    Q = pp.tile([128, NT], f32, tag="Q", name=f"Q{t}_{m}")
                    V = pp.tile([128, NT], f32, tag="V", name=f"V{t}_{m}")
                    W = pp.tile([128, NT], f32, tag="W", bufs=1, name=f"W{t}_{m}")
                    for p in range(4):
                        rhsE = se_c[t][:, p : p + NT]
                        rhsO = so_c[t][:, p : p + NT]
                        st = dict(start=(p == 0), stop=(p == 3))
                        nc.tensor.matmul(P[:], lhsT(m, p, 0), rhsE, **st)
                        nc.tensor.matmul(Q[:], lhsT(m, p, 1), rhsO, **st)
                        nc.tensor.matmul(V[:], lhsT(m, p, 2), rhsE, **st)
                        nc.tensor.matmul(W[:], lhsT(m, p, 3), rhsO, **st)
                    nc.scalar.copy(qs[:, sl], Q[:])
                    nc.scalar.copy(ws[:, sl], W[:])
                    nc.vector.scalar_tensor_tensor(
                        u[:, sl], P[:], 0.0, qs[:, sl], BP, ADD
                    )
                    nc.vector.scalar_tensor_tensor(
                        un[:, sl], P[:], -1.0, qs[:, sl], MUL, ADD
                    )
                    nc.vector.scalar_tensor_tensor(
                        v[:, sl], V[:], 0.0, ws[:, sl], BP, ADD
                    )
                    nc.vector.scalar_tensor_tensor(
                        vn[:, sl], V[:], -1.0, ws[:, sl], MUL, ADD
                    )
                # A side (bins 1..256): mag = sqrt((P+Q)^2 + (V+W)^2)
                # B side (bins 511..257): mag = sqrt((Q-P)^2 + (W-V)^2)
                s1q.append((u, v, un, vn, t0))
                if len(s1q) > 1:
                    _emit_s1(s1q.pop(0))
                if len(s2q) > 1:
                    _emit_s2(s2q.pop(0))
                if len(s3q) > 1:
                    _emit_s3(s3q.pop(0))
                if len(s4q) > 1:
                    _emit_s4(s4q.pop(0))

            for _ in range(N_TILES):
                if s1q:
                    _emit_s1(s1q.pop(0))
                if s2q:
                    _emit_s2(s2q.pop(0))
                if s3q:
                    _emit_s3(s3q.pop(0))
                if s4q:
                    _emit_s4(s4q.pop(0))

            outln = const.tile([80, F], f32)
            H = F // 4
            for h in range(4):
                nc.scalar.activation(
                    outln[:, h * H : (h + 1) * H], outbuf[:, h * H : (h + 1) * H],
                    mybir.ActivationFunctionType.Ln, bias=1.0, scale=10000.0,
                )
                nc.sync.dma_start(
                    out=out_d[:, h * H : (h + 1) * H],
                    in_=outln[:, h * H : (h + 1) * H],
                )
    _split_sync_waits(nc)
    return nc


def _prep_inputs(audio, basis_r, basis_i, mel_basis):
    audio = np.asarray(audio, dtype=np.float32)
    basis_r = np.asarray(basis_r, dtype=np.float32)
    basis_i = np.asarray(basis_i, dtype=np.float32)
    mel_basis = np.asarray(mel_basis, dtype=np.float32)

    Ce = basis_r[:257, 0::2]
    Co = basis_r[:257, 1::2]
    Se = basis_i[:257, 0::2]
    So = basis_i[:257, 1::2]
    mats = (Ce, Co, Se, So)
    wts = np.empty((128, 32 * 128), dtype=NP_DT)
    for m in range(2):
        for p in range(4):
            for x in range(4):
                q = (m * 16 + p * 4 + x) * 128
                blk = mats[x][1 + m * 128 : 1 + (m + 1) * 128, p * 128 : (p + 1) * 128]
                wts[:, q : q + 128] = blk.T.astype(NP_DT)

    # mel matrix, bins permuted to the device's mag row order:
    # A side rows = bins 1..256; B side row j = bin 511-j (row 255 is the
    # bin-256 duplicate -> zeroed)
    melA = mel_basis[:, 1:257]
    melB = mel_basis[:, [511 - j for j in range(256)]].copy()
    melB[:, 255] = 0.0
    melt = np.empty((128, 4 * 80), dtype=NP_DT)
    melt[:, 0:80] = melA[:, 0:128].T.astype(NP_DT)
    melt[:, 80:160] = melA[:, 128:256].T.astype(NP_DT)
    melt[:, 160:240] = melB[:, 0:128].T.astype(NP_DT)
    melt[:, 240:320] = melB[:, 128:256].T.astype(NP_DT)

    in_maps = []
    for b in range(N_CORES):
        row = audio[b]
        x = np.concatenate([row[PAD:0:-1], row, row[-2 : -PAD - 2 : -1]])
        se = np.ascontiguousarray(x[0::2].reshape(SEG, 128).T).astype(NP_DT)
        so = np.ascontiguousarray(x[1::2].reshape(SEG, 128).T).astype(NP_DT)
        in_maps.append({"se": se, "so": so, "wts": wts, "melt": melt})
    return in_maps


def kernel(audio, basis_r, basis_i, mel_basis):
    if "nc" not in _cache:
        _cache["nc"] = _build_program()
    nc = _cache["nc"]
    in_maps = _prep_inputs(audio, basis_r, basis_i, mel_basis)

    trace = os.environ.get("MELSPEC_TRACE") == "1"
    if trace:
        import types
        import trn_agent_boot.trn_boot as tb
        import concourse.bass_utils as bu

        if "antenv.axon_hooks" not in sys.modules:
            hook = tb._ntff_profile_via_ctypes("/opt/axon/libaxon_pjrt.so")
            mod = types.ModuleType("antenv.axon_hooks")
            mod.get_axon_ntff_profile_hook = lambda: hook
            sys.modules["antenv.axon_hooks"] = mod
        bu.upload_artifacts = lambda tmpdir: f"local://{tmpdir}"

    res = run_bass_kernel_spmd(nc, in_maps, list(range(N_CORES)), trace=trace)
    _cache["last_results"] = res
    out = np.stack([res.results[i]["out"] for i in range(N_CORES)])
    return out.astype(np.float32)


# revision 41
# speedup vs baseline: 1.1166x; 1.0058x over previous
"""MelSpectrogram Trainium2 kernel.

Full inputs in, full output out. Data-parallel over batch B=8 across the
8 NeuronCores (one audio row per core); DFT basis and mel filterbank are
replicated (prepped host-side into matmul-friendly layouts).

Per-core device algorithm (one audio row, T=1048576):

The reflect-padded signal x (len 4099*256) is split host-side into even/
odd sample streams laid out as SBUF tensors SE/SO [128, 4099] with
SE[l,j] = x[2*(j*128+l)], SO[l,j] = x[2*(j*128+l)+1]. Because HOP=256
divides FILTER_LEN=1024, frame f's even-tap block p (taps 2*(p*128+l))
is SE[:, f+p] -- the windowed DFT needs no frame materialization, just
shifted rhs slices.

Radix-2 bin pairing: the DFT basis satisfies basis[512-k, n] =
+/-(-1)^n basis[k, n], so with P = Ce@xe, Q = Co@xo, V = Se@xe,
W = So@xo (even/odd tap split of the windowed cos/sin bases, bin rows
1..256):
    mag[k]      = sqrt((P+Q)^2 + (V+W)^2)   k = 1..256
    mag[512-k]  = sqrt((P-Q)^2 + (V-W)^2)   k = 1..255
Mel filter weights at bins 0 and 512 are exactly zero (fmin=0,
fmax=sr/2 edge filters), so bins 1..511 cover everything: the DFT
matmul work halves versus the direct form, in clean 2x128-row tiles
(the bin-256 duplicate on the B side is zeroed in the permuted mel
matrix host-side).

magnitudes feed a PSUM-accumulated mel projection on PE; final
log1p(1e4*x) = Ln(1e4*x + 1) on ACT, streamed against the output DMA.
"""

import os
import sys

sys.path.insert(0, "/opt/trn_rl_repo")

import numpy as np
import concourse.bass as bass
import concourse.mybir as mybir
import concourse.tile as tile
from concourse.bass_utils import run_bass_kernel_spmd
from concourse.vector_clock import ScopedClock

N_CORES = 8
T = 1048576
PAD = 384
SEG = 4099  # (T + 2*PAD) / 256
F = 4096  # output frames
NT = 512  # frames per tile
N_TILES = F // NT
DT = mybir.dt.float16
NP_DT = np.float16

_cache = {}


class _PatchedTileContext(tile.TileContext):
    # This walrus build rejects >1 sync-wait per instruction (and any on
    # the kernel-tail Drain): carry the global-clock waits on
    # one-NoOp-per-wait ahead of the drain instead.
    def _drain_and_barrier(self, tick_clock, wait_clock):
        nop_inst = self.nc.sync.nop(nofuse=True, hint="pre_drain_waits")
        wait_clock.add_sem_waits(
            nop_inst.ins, ScopedClock({None: tick_clock.global_clock})
        )
        waits = list(nop_inst.ins.sync_info.on_wait)
        if len(waits) > 1:
            si = nop_inst.ins.sync_info
            si.on_wait = waits[:1]
            nop_inst.ins.sync_info = si
            for w in waits[1:]:
                extra = self.nc.sync.nop(nofuse=True, hint="pre_drain_waits")
                esi = extra.ins.sync_info or mybir.SyncInfo(on_wait=[], on_update=[])
                esi.on_wait = [w]
                extra.ins.sync_info = esi
        self.nc.sync.drain()
        self.nc.all_engine_barrier()
        assert self.sems is not None
        popped = self.nc._tile_sem_poison_stack.pop()
        assert popped is self._sem_poison
        self.nc.clear_and_free_semaphores(list(self.sems.allocated().values()))
        self.nc.all_engine_barrier()


def _split_sync_waits(nc, cap=1):
    # Hoist excess sync-waits onto same-engine NoOps placed just before
    # the instruction (engines are in-order, so semantics preserved).
    for f in nc.m.functions:
        for bb in f.blocks:
            out = []
            changed = False
            for inst in bb.instructions:
                si = inst.sync_info
                waits = list(si.on_wait) if si else []
                if len(waits) > cap:
                    changed = True
                    for w in waits[:-cap]:
                        nop = mybir.InstNoOp(
                            name=nc.get_next_instruction_name(), ins=[], outs=[]
                        )
                        nop.engine = inst.engine
                        nop.sync_info = mybir.SyncInfo(on_wait=[w], on_update=[])
                        out.append(nop)
                    si.on_wait = waits[-cap:]
                    inst.sync_info = si
                out.append(inst)
            if changed:
                bb.instructions = out


def _build_program():
    nc = bass.Bass()
    se_d = nc.dram_tensor("se", [128, SEG], DT, kind="ExternalInput")
    so_d = nc.dram_tensor("so", [128, SEG], DT, kind="ExternalInput")
    # 32 blocks of [128,128]: q = m*16 + p*4 + x, x in (Ce,Co,Se,So)
    wts_d = nc.dram_tensor("wts", [128, 32 * 128], DT, kind="ExternalInput")
    melt_d = nc.dram_tensor("melt", [128, 4 * 80], DT, kind="ExternalInput")
    out_d = nc.dram_tensor("out", [80, F], mybir.dt.float32, kind="ExternalOutput")

    f32 = mybir.dt.float32
    NT2 = 2 * NT
    with _PatchedTileContext(nc) as tc:
        with (
            tc.tile_pool(name="const", bufs=1) as const,
            tc.tile_pool(name="work", bufs=2) as work,
            tc.tile_pool(name="psum", bufs=1, space="PSUM") as pp,
        ):
            se_c, so_c = [], []
            for t in range(N_TILES):
                c0 = t * NT
                c1 = min(SEG, (t + 1) * NT + 3)
                se = const.tile([128, c1 - c0], DT, tag=f"se{t}", name=f"se{t}")
                so = const.tile([128, c1 - c0], DT, tag=f"so{t}", name=f"so{t}")
                se_c.append(se)
                so_c.append(so)
            # weights split per (m, p) so matmuls gate on 128KB pieces that
            # stream in program order down SP's FIFO DMA queue
            wp = [
                const.tile([128, 512], DT, tag=f"wp{j}", name=f"wp{j}")
                for j in range(8)
            ]
            melt_sb = const.tile([128, 4 * 80], DT)

            # SP FIFO delivers in emission order: first-tile criticals,
            # then the rest of the weights, then audio chunks in use order.
            nc.sync.dma_start(out=wp[0][:], in_=wts_d[:, 0:512])
            nc.sync.dma_start(out=se_c[0][:], in_=se_d[:, 0 : NT + 3])
            nc.sync.dma_start(out=so_c[0][:], in_=so_d[:, 0 : NT + 3])
            for j in range(1, 8):
                nc.sync.dma_start(out=wp[j][:], in_=wts_d[:, j * 512 : (j + 1) * 512])
            nc.sync.dma_start(out=melt_sb[:], in_=melt_d[:])
            for t in range(1, N_TILES):
                c0 = t * NT
                c1 = min(SEG, (t + 1) * NT + 3)
                nc.sync.dma_start(out=se_c[t][:], in_=se_d[:, c0:c1])
                nc.sync.dma_start(out=so_c[t][:], in_=so_d[:, c0:c1])
            outbuf = const.tile([80, F], f32)

            def lhsT(m, p, x):
                return wp[m * 4 + p][:, x * 128 : (x + 1) * 128]

            BP = mybir.AluOpType.bypass
            ADD = mybir.AluOpType.add
            SUB = mybir.AluOpType.subtract
            MUL = mybir.AluOpType.mult

            # Software pipeline: engines have strict in-order queues, so an
            # op whose input is produced in the same tile stalls its whole
            # queue. Emitting each stage one tile later than its producer
            # ensures inputs are a full tile old by the time the engine
            # reaches the op. Stages:
            #   S0(t): DFT matmuls + PSUM-freeing copies/combines
            #   S1(t-1): squares   S2(t-2): adds   S3(t-3): sqrts
            #   S4(t-4): mel matmuls + outbuf copy
            s1q, s2q, s3q, s4q = [], [], [], []

            # HAM warmup: dummy matmuls on uninitialized SBUF (no DMA
            # deps, output never read) keep the PE busy while the input
            # DMAs stream, so the real matmuls start at 2.4GHz instead of
            # paying the ~3.4us cold-clock window
            warmsrc = const.tile([128, NT], DT, name="warmsrc")
            nc.gpsimd.memset(warmsrc[:], 0)
            warm_ps = pp.tile([80, NT], f32, tag="mel", bufs=1, name="warm")
            for _ in range(12):
                nc.tensor.matmul(
                    warm_ps[:], warmsrc[:, :80], warmsrc[:], start=True, stop=True
                )

            def _emit_s1(it):
                u, v, un, vn, t0p = it
                a = work.tile([128, NT2], DT, tag="a", bufs=3, name=f"a{t0p}")
                nc.scalar.square(a[:], u[:])
                b = work.tile([128, NT2], DT, tag="b", bufs=3, name=f"b{t0p}")
                nc.scalar.square(b[:], v[:])
                a2 = work.tile([128, NT2], DT, tag="a2", bufs=3, name=f"a2_{t0p}")
                nc.vector.scalar_tensor_tensor(a2[:], un[:], 1.0, un[:], MUL, MUL)
                b2 = work.tile([128, NT2], DT, tag="b2", bufs=3, name=f"b2_{t0p}")
                nc.gpsimd.tensor_tensor(b2[:], vn[:], vn[:], MUL)
                s2q.append((a, b, a2, b2, t0p))

            def _emit_s2(it):
                a, b, a2, b2, t0p = it
                m2A = work.tile([128, NT2], DT, tag="m2A", bufs=3, name=f"m2A{t0p}")
                nc.gpsimd.tensor_tensor(m2A[:], a[:], b[:], ADD)
                m2B = work.tile([128, NT2], DT, tag="m2B", bufs=3, name=f"m2B{t0p}")
                nc.gpsimd.tensor_tensor(m2B[:], a2[:], b2[:], ADD)
                s3q.append((m2A, m2B, t0p))

            def _emit_s3(it):
                m2A, m2B, t0p = it
                magA = work.tile([128, NT2], DT, tag="magA", bufs=3, name=f"mgA{t0p}")
                nc.scalar.sqrt(magA[:], m2A[:])
                magB = work.tile([128, NT2], DT, tag="magB", bufs=3, name=f"mgB{t0p}")
                nc.scalar.sqrt(magB[:], m2B[:])
                s4q.append((magA, magB, t0p))

            def _emit_s4(it):
                mA, mB, t0p = it
                mel_ps = pp.tile([80, NT], f32, tag="mel", bufs=1, name=f"mel{t0p}")
                rhss = [
                    mA[:, 0:NT], mA[:, NT:NT2],
                    mB[:, 0:NT], mB[:, NT:NT2],
                ]
                for i, rhs in enumerate(rhss):
                    nc.tensor.matmul(
                        mel_ps[:], melt_sb[:, i * 80 : (i + 1) * 80], rhs,
                        start=(i == 0), stop=(i == 3),
                    )
                nc.vector.tensor_copy(out=outbuf[:, t0p : t0p + NT], in_=mel_ps[:])

            for t in range(N_TILES):
                t0 = t * NT
                # narrow per-m PSUM tiles (P,Q,V double-buffered; W and mel
                # single); wide SBUF intermediates written in m-halves
                qs = work.tile([128, NT2], DT, tag="qs", bufs=4)
                ws = work.tile([128, NT2], DT, tag="ws", bufs=4)
                u = work.tile([128, NT2], DT, tag="u", bufs=4)
                un = work.tile([128, NT2], DT, tag="un", bufs=4)
                v = work.tile([128, NT2], DT, tag="v", bufs=4)
                vn = work.tile([128, NT2], DT, tag="vn", bufs=4)
                for m in range(2):
                    sl = slice(m * NT, (m + 1) * NT)
                    P = pp.tile([128, NT], f32, tag="P", name=f"P{t}_{m}")
                    Q = pp.tile([128, NT], f32, tag="Q", name=f"Q{t}_{m}")
                    V = pp.tile([128, NT], f32, tag="V", name=f"V{t}_{m}")
                    W = pp.tile([128, NT], f32, tag="W", bufs=1, name=f"W{t}_{m}")
                    for p in range(4):
                        rhsE = se_c[t][:, p : p + NT]
                        rhsO = so_c[t][:, p : p + NT]
                        st = dict(start=(p == 0), stop=(p == 3))
                        nc.tensor.matmul(P[:], lhsT(m, p, 0), rhsE, **st)
                        nc.tensor.matmul(Q[:], lhsT(m, p, 1), rhsO, **st)
                        nc.tensor.matmul(V[:], lhsT(m, p, 2), rhsE, **st)
                        nc.tensor.matmul(W[:], lhsT(m, p, 3), rhsO, **st)
                    nc.scalar.copy(qs[:, sl], Q[:])
                    nc.vector.tensor_copy(out=ws[:, sl], in_=W[:])
                    nc.vector.scalar_tensor_tensor(
                        u[:, sl], P[:], 0.0, qs[:, sl], BP, ADD
                    )
                    nc.vector.scalar_tensor_tensor(
                        un[:, sl], P[:], -1.0, qs[:, sl], MUL, ADD
                    )
                    nc.vector.scalar_tensor_tensor(
                        v[:, sl], V[:], 0.0, ws[:, sl], BP, ADD
                    )
                    nc.vector.scalar_tensor_tensor(
                        vn[:, sl], V[:], -1.0, ws[:, sl], MUL, ADD
                    )
                # A side (bins 1..256): mag = sqrt((P+Q)^2 + (V+W)^2)
                # B side (bins 511..257): mag = sqrt((Q-P)^2 + (W-V)^2)
                s1q.append((u, v, un, vn, t0))
                if len(s1q) > 1:
                    _emit_s1(s1q.pop(0))
                if len(s2q) > 1:
                    _emit_s2(s2q.pop(0))
                if len(s3q) > 1:
                    _emit_s3(s3q.pop(0))
                if len(s4q) > 1:
                    _emit_s4(s4q.pop(0))

            for _ in range(N_TILES):
                if s1q:
                    _emit_s1(s1q.pop(0))
                if s2q:
                    _emit_s2(s2q.pop(0))
                if s3q:
                    _emit_s3(s3q.pop(0))
                if s4q:
                    _emit_s4(s4q.pop(0))

            outln = const.tile([80, F], f32)
            H = F // 4
            for h in range(4):
                nc.scalar.activation(
                    outln[:, h * H : (h + 1) * H], outbuf[:, h * H : (h + 1) * H],
                    mybir.ActivationFunctionType.Ln, bias=1.0, scale=10000.0,
                )
                nc.sync.dma_start(
                    out=out_d[:, h * H : (h + 1) * H],
                    in_=outln[:, h * H : (h + 1) * H],
                )
    _split_sync_waits(nc)
    return nc


def _prep_inputs(audio, basis_r, basis_i, mel_basis):
    audio = np.asarray(audio, dtype=np.float32)
    basis_r = np.asarray(basis_r, dtype=np.float32)
    basis_i = np.asarray(basis_i, dtype=np.float32)
    mel_basis = np.asarray(mel_basis, dtype=np.float32)

    Ce = basis_r[:257, 0::2]
    Co = basis_r[:257, 1::2]
    Se = basis_i[:257, 0::2]
    So = basis_i[:257, 1::2]
    mats = (Ce, Co, Se, So)
    wts = np.empty((128, 32 * 128), dtype=NP_DT)
    for m in range(2):
        for p in range(4):
            for x in range(4):
                q = (m * 16 + p * 4 + x) * 128
                blk = mats[x][1 + m * 128 : 1 + (m + 1) * 128, p * 128 : (p + 1) * 128]
                wts[:, q : q + 128] = blk.T.astype(NP_DT)

    # mel matrix, bins permuted to the device's mag row order:
    # A side rows = bins 1..256; B side row j = bin 511-j (row 255 is the
    # bin-256 duplicate -> zeroed)
    melA = mel_basis[:, 1:257]
    melB = mel_basis[:, [511 - j for j in range(256)]].copy()
    melB[:, 255] = 0.0
    melt = np.empty((128, 4 * 80), dtype=NP_DT)
    melt[:, 0:80] = melA[:, 0:128].T.astype(NP_DT)
    melt[:, 80:160] = melA[:, 128:256].T.astype(NP_DT)
    melt[:, 160:240] = melB[:, 0:128].T.astype(NP_DT)
    melt[:, 240:320] = melB[:, 128:256].T.astype(NP_DT)

    in_maps = []
    for b in range(N_CORES):
        row = audio[b]
        x = np.concatenate([row[PAD:0:-1], row, row[-2 : -PAD - 2 : -1]])
        se = np.ascontiguousarray(x[0::2].reshape(SEG, 128).T).astype(NP_DT)
        so = np.ascontiguousarray(x[1::2].reshape(SEG, 128).T).astype(NP_DT)
        in_maps.append({"se": se, "so": so, "wts": wts, "melt": melt})
    return in_maps


def kernel(audio, basis_r, basis_i, mel_basis):
    if "nc" not in _cache:
        _cache["nc"] = _build_program()
    nc = _cache["nc"]
    in_maps = _prep_inputs(audio, basis_r, basis_i, mel_basis)

    trace = os.environ.get("MELSPEC_TRACE") == "1"
    if trace:
        import types
        import trn_agent_boot.trn_boot as tb
        import concourse.bass_utils as bu

        if "antenv.axon_hooks" not in sys.modules:
            hook = tb._ntff_profile_via_ctypes("/opt/axon/libaxon_pjrt.so")
            mod = types.ModuleType("antenv.axon_hooks")
            mod.get_axon_ntff_profile_hook = lambda: hook
            sys.modules["antenv.axon_hooks"] = mod
        bu.upload_artifacts = lambda tmpdir: f"local://{tmpdir}"

    res = run_bass_kernel_spmd(nc, in_maps, list(range(N_CORES)), trace=trace)
    _cache["last_results"] = res
    out = np.stack([res.results[i]["out"] for i in range(N_CORES)])
    return out.astype(np.float32)
